# revision 12
# baseline (speedup 1.0000x reference)
"""Trainium2 Bass kernel for masked (block-sparse) attention.

Problem: B=2, H=16, Q=2304, KV=2305, D=64, fp32, with a static structured mask
(policy-transformer causal mask: 8 steps x 288 tokens, 1 sink token, history=4).

Sharding: the 32 (b,h) pairs are split 4-per-core across 8 NeuronCores (pure
data/head parallelism, no collectives).

Per-core algorithm (per head):
  - q/k are transposed on-device (PE transpose) into d-major layout.
  - The mask is folded INTO the QK^T matmul as 24 extra contraction rows:
    mask bias depends on q only through (token-class, step) -- 3 classes x 8
    steps -- so  bias[kv,q] = u[:, kv].T @ w[:, q]  exactly, with u/w small
    host-precomputed tables appended below k^T / q^T.  Masked scores come out
    of the matmul at -240000 and exp(0.125*s) flushes them to exactly 0.
  - S^T chunks [128 kv x 256 q] are computed only for kv-chunks inside each
    q-block's history window (block-sparse skip), exp'd on the scalar engine
    (scale=0.125 applies 1/sqrt(64)), and PV accumulates out^T = [V|1]^T @ P^T
    in PSUM; the appended ones-column of V yields the softmax denominator.
  - Finalize: transpose out^T back with the PE, multiply by the reciprocal
    denominator, DMA out.  No max-subtraction is needed (|scores| <= ~8).
"""

import numpy as np

import concourse.bacc as bacc
import concourse.bass as bass
import concourse.tile as tile
from concourse import mybir
from concourse.bass_utils import run_bass_kernel_spmd

# ---- problem constants -------------------------------------------------------
B, H, D = 2, 16, 64
N_STEPS, N_SINK, HIST = 8, 1, 4
ONE_STEP = 288
TOK_TO_AO = 280
Q_LEN = ONE_STEP * N_STEPS          # 2304
KV_LEN = Q_LEN + N_SINK             # 2305
KV_PAD = 2432                       # 19 * 128
BIG = 30000.0 * 8.0                 # pre-scale mask bias (ACT applies 0.125)
QBLK = 256
CHUNK = 128
N_QBLK = Q_LEN // QBLK              # 9
HEADS_PER_CORE = 4
N_CORES = 8
GROUP = 4                           # S^T chunks per PSUM group / exp call
FP32 = mybir.dt.float32
F32R = mybir.dt.float32r
import os
USE_F32R = os.environ.get("ATTN_F32R", "1") == "1"  # single-pass reduced-precision matmuls
MMDT_G = F32R if USE_F32R else FP32


# ---- host-side mask tables ---------------------------------------------------
def _build_mask():
    q_idx = np.arange(Q_LEN)[:, None]
    kv_idx = np.arange(KV_LEN)[None, :]
    is_sink = kv_idx < N_SINK
    kv_b = kv_idx - N_SINK

    def classify(idx):
        r = idx % ONE_STEP
        return r < TOK_TO_AO, r > TOK_TO_AO, r == TOK_TO_AO, idx // ONE_STEP

    q_img, q_ra, q_ao, q_step = classify(q_idx)
    kv_img, kv_ra, kv_ao, kv_step = classify(kv_b)
    prev = kv_step < q_step
    same = kv_step == q_step
    full = (
        (q_img & ~kv_ao & prev)
        | (q_ao & ~kv_ao & prev)
        | (q_ra & ~kv_ao & prev)
        | (q_img & kv_img & same)
        | (q_ao & (kv_img | kv_ao) & same)
        | (q_ra & ~kv_ao & same)
    )
    full = full & (q_step - kv_step <= HIST)
    return is_sink | full


def _build_uw():
    mask = _build_mask()
    w = np.zeros((24, Q_LEN), np.float32)
    u = np.zeros((24, KV_PAD), np.float32)
    q_r = np.arange(Q_LEN) % ONE_STEP
    q_s = np.arange(Q_LEN) // ONE_STEP
    cls = np.where(q_r < TOK_TO_AO, 0, np.where(q_r == TOK_TO_AO, 1, 2))
    for s in range(N_STEPS):
        for c in range(3):
            row = s * 3 + c
            w[row] = ((q_s == s) & (cls == c)).astype(np.float32)
            rep = s * ONE_STEP + (0, TOK_TO_AO, TOK_TO_AO + 1)[c]
            u[row, :KV_LEN] = np.where(mask[rep], 0.0, -BIG)
            u[row, KV_LEN:] = -BIG
    return u, w


def _chunk_lists():
    blocks = []
    for b in range(N_QBLK):
        r0, r1 = b * QBLK, b * QBLK + QBLK - 1
        s0, s1 = r0 // ONE_STEP, r1 // ONE_STEP
        ws = max(0, s0 - HIST)
        end = ONE_STEP * (s1 + 1) + 1
        c_hi = (end + CHUNK - 1) // CHUNK - 1
        if ws == 0:
            chunks = list(range(0, c_hi + 1))
        else:
            chunks = [0] + list(range((ONE_STEP * ws + 1) // CHUNK, c_hi + 1))
        blocks.append(chunks)
    return blocks


# ---- bass kernel builder -----------------------------------------------------
def _emit(nc, tc, q_d, k_d, v_d, o_d, u_d, w_d, id_d):
    Exp = mybir.ActivationFunctionType.Exp
    blocks = _chunk_lists()

    MMDT = F32R if USE_F32R else FP32

    def mm(out, lhsT, rhs, **kw):
        nc.tensor.matmul(out, lhsT, rhs, **kw)

    import contextlib
    with contextlib.ExitStack() as ctx:
        const_p = ctx.enter_context(tc.tile_pool(name="const", bufs=1))
        big_p = ctx.enter_context(tc.tile_pool(name="big", bufs=2))
        stage_p = ctx.enter_context(tc.tile_pool(name="stage", bufs=2))
        pt_p = ctx.enter_context(tc.tile_pool(name="pt", bufs=3))
        fin_p = ctx.enter_context(tc.tile_pool(name="fin", bufs=2))
        ps_s = ctx.enter_context(tc.tile_pool(name="ps_s", bufs=2, space="PSUM"))
        ps_pv = ctx.enter_context(tc.tile_pool(name="ps_pv", bufs=2, space="PSUM"))
        ps_tr = ctx.enter_context(tc.tile_pool(name="ps_tr", bufs=2, space="PSUM"))

        ident = const_p.tile([128, 128], FP32)
        nc.sync.dma_start(ident[:], id_d.ap()[:])

        def emit_loads(h):
            stq = stage_p.tile([128, 18 * 64], FP32, tag="stq", name=f"stq{h}")
            nc.sync.dma_start(
                stq[:].rearrange("p (t d) -> p t d", d=64),
                q_d.ap()[h].rearrange("(t p) d -> p t d", p=128))
            stk = stage_p.tile([128, 19 * 64], FP32, tag="stk", name=f"stk{h}")
            nc.sync.dma_start(
                stk[:, 0:18 * 64].rearrange("p (t d) -> p t d", d=64),
                k_d.ap()[h, 0:2304].rearrange("(t p) d -> p t d", p=128))
            nc.sync.dma_start(stk[0:1, 18 * 64:19 * 64], k_d.ap()[h, 2304:2305, :])

            qT = big_p.tile([88, Q_LEN], MMDT, tag="qT", name=f"qT{h}")
            kT = big_p.tile([88, KV_PAD], MMDT, tag="kT", name=f"kT{h}")
            nc.sync.dma_start(qT[64:88, :], w_d.ap()[:])
            nc.sync.dma_start(kT[64:88, :], u_d.ap()[:])
            nc.vector.memset(kT[0:64, KV_LEN:KV_PAD].bitcast(FP32), 0.0)

            vsb = big_p.tile([128, 19 * 65], MMDT, tag="vsb", name=f"vsb{h}")
            va = v_d.ap()[h]
            nc.vector.memset(vsb[:, 18 * 65:18 * 65 + 64].bitcast(FP32), 0.0)
            nc.sync.dma_start(
                vsb[:].rearrange("p (c x) -> p c x", x=65)[:, 0:18, 0:64],
                va[0:2304].rearrange("(c p) d -> p c d", p=128))
            nc.sync.dma_start(vsb[0:1, 18 * 65:18 * 65 + 64], va[2304:2305, :])
            ones_view = vsb[:].rearrange("p (c x) -> p c x", x=65)[:, :, 64:65]
            nc.vector.memset(ones_view.bitcast(FP32), 1.0)
            return stq, stk, qT, kT, vsb

        # PE-transpose PAIRS of 128x64 tiles in one [128,128] is_transpose
        # (two pairs per PSUM tile); copies split across ACT and DVE so the
        # PSUM slot frees quickly.
        def load_T2(st, dst, pair_ids):
            tr = ps_tr.tile([128, 256], FP32, tag="tr")
            for j, g in enumerate(pair_ids):
                nc.tensor.transpose(tr[:, j * 128:(j + 1) * 128],
                                    st[:, g * 128:(g + 1) * 128],
                                    ident[:])
            for j, g in enumerate(pair_ids):
                nc.scalar.copy(
                    dst[0:64, (2 * g) * 128:(2 * g) * 128 + 128],
                    tr[0:64, j * 128:(j + 1) * 128])
                nc.vector.tensor_copy(
                    dst[0:64, (2 * g + 1) * 128:(2 * g + 1) * 128 + 128],
                    tr[64:128, j * 128:(j + 1) * 128])

        def transpose_units(tiles):
            stq, stk, qT, kT, vsb = tiles
            units = []
            for g in range(0, 9, 2):
                units.append(lambda g=g: load_T2(stq, qT, [g] if g == 8 else [g, g + 1]))
            for g in range(0, 9, 2):
                units.append(lambda g=g: load_T2(stk, kT, [g] if g == 8 else [g, g + 1]))

            def tiny():
                trl = ps_tr.tile([128, 256], FP32, tag="tr")
                nc.tensor.transpose(trl[0:64, 0:1], stk[0:1, 18 * 64:19 * 64],
                                    ident[0:1, 0:1])
                nc.vector.tensor_copy(kT[0:64, 2304:2305], trl[0:64, 0:1])
            units.append(tiny)
            return units

        def emit_block(h, b, chunks, tiles, osb):
            stq, stk, qT, kT, vsb = tiles
            qTb = qT[:, b * QBLK:(b + 1) * QBLK]
            pv = ps_pv.tile([65, QBLK], FP32, tag="pv")
            n = len(chunks)
            first = True
            for g0 in range(0, n, GROUP):
                grp = chunks[g0:g0 + GROUP]
                sg = ps_s.tile([128, GROUP * QBLK], FP32, tag="sg")
                for j, c in enumerate(grp):
                    mm(sg[:, j * QBLK:(j + 1) * QBLK],
                       kT[:, c * CHUNK:(c + 1) * CHUNK],
                       qTb, start=True, stop=True)
                pt = pt_p.tile([128, GROUP * QBLK], MMDT, tag="pt")
                gw = len(grp) * QBLK
                nc.scalar.activation(pt[:, 0:gw], sg[:, 0:gw], Exp, scale=0.125)
                for j, c in enumerate(grp):
                    last = (g0 + j + 1 == n)
                    mm(pv[:], vsb[:, c * 65:(c + 1) * 65],
                       pt[:, j * QBLK:(j + 1) * QBLK],
                       start=first, stop=last, skip_group_check=True)
                    first = False

            pvs = fin_p.tile([65, QBLK], FP32, tag="pvs")
            nc.vector.tensor_copy(pvs[:], pv[:])
            tf = ps_tr.tile([128, 130], FP32, tag="tr")
            nc.tensor.transpose(tf[:, 0:65], pvs[:, 0:128], ident[0:65, 0:65])
            nc.tensor.transpose(tf[:, 65:130], pvs[:, 128:256], ident[0:65, 0:65])
            rcp = fin_p.tile([128, 2], FP32, tag="rcp")
            nc.vector.reciprocal(rcp[:, 0:1], tf[:, 64:65])
            nc.vector.reciprocal(rcp[:, 1:2], tf[:, 129:130])
            nc.vector.tensor_scalar_mul(osb[:, b * 128:b * 128 + 64],
                                        tf[:, 0:64], rcp[:, 0:1])
            nc.vector.tensor_scalar_mul(osb[:, b * 128 + 64:b * 128 + 128],
                                        tf[:, 65:129], rcp[:, 1:2])

        # software-pipelined head loop: head h+1's loads/transposes are
        # interleaved between head h's blocks so its setup hides under
        # chunk compute.
        tiles = emit_loads(0)
        for unit in transpose_units(tiles):
            unit()
        for h in range(HEADS_PER_CORE):
            nxt_units = []
            if h + 1 < HEADS_PER_CORE:
                nxt_tiles = emit_loads(h + 1)
                nxt_units = transpose_units(nxt_tiles)
            osb = fin_p.tile([128, 18 * 64], FP32, tag="osb", name=f"osb{h}")
            per = max(1, (len(nxt_units) + N_QBLK - 1) // N_QBLK)
            for b, chunks in enumerate(blocks):
                emit_block(h, b, chunks, tiles, osb)
                for _ in range(per):
                    if nxt_units:
                        nxt_units.pop(0)()
            while nxt_units:
                nxt_units.pop(0)()
            nc.sync.dma_start(
                o_d.ap()[h].rearrange("(t p) d -> p t d", p=128),
                osb[:].rearrange("p (t d) -> p t d", d=64))
            if h + 1 < HEADS_PER_CORE:
                tiles = nxt_tiles


_CACHE = {}


def _get_nc():
    if "nc" not in _CACHE:
        nc = bacc.Bacc("TRN2", target_bir_lowering=False, debug=False)
        q_d = nc.dram_tensor("q", [HEADS_PER_CORE, Q_LEN, D], FP32, kind="ExternalInput")
        k_d = nc.dram_tensor("k", [HEADS_PER_CORE, KV_LEN, D], FP32, kind="ExternalInput")
        v_d = nc.dram_tensor("v", [HEADS_PER_CORE, KV_LEN, D], MMDT_G, kind="ExternalInput")
        u_d = nc.dram_tensor("utab", [24, KV_PAD], MMDT_G, kind="ExternalInput")
        w_d = nc.dram_tensor("wtab", [24, Q_LEN], MMDT_G, kind="ExternalInput")
        id_d = nc.dram_tensor("ident", [128, 128], FP32, kind="ExternalInput")
        o_d = nc.dram_tensor("o", [HEADS_PER_CORE, Q_LEN, D], FP32, kind="ExternalOutput")
        with tile.TileContext(nc) as tc:
            _emit(nc, tc, q_d, k_d, v_d, o_d, u_d, w_d, id_d)
        nc.compile()
        _CACHE["nc"] = nc
    return _CACHE["nc"]


LAST_RESULT = None


def kernel(q, k, v):
    global LAST_RESULT
    q = np.ascontiguousarray(np.asarray(q, np.float32).reshape(B * H, Q_LEN, D))
    k = np.ascontiguousarray(np.asarray(k, np.float32).reshape(B * H, KV_LEN, D))
    v = np.ascontiguousarray(np.asarray(v, np.float32).reshape(B * H, KV_LEN, D))

    u, w = _build_uw()
    ident = np.eye(128, dtype=np.float32)

    nc = _get_nc()
    in_maps = []
    for c in range(N_CORES):
        s = slice(c * HEADS_PER_CORE, (c + 1) * HEADS_PER_CORE)
        in_maps.append({
            "q": q[s], "k": k[s], "v": v[s],
            "utab": u, "wtab": w, "ident": ident,
        })
    res = run_bass_kernel_spmd(nc, in_maps, list(range(N_CORES)))
    LAST_RESULT = res
    out = np.concatenate([res.results[c]["o"] for c in range(N_CORES)], axis=0)
    return out.reshape(B, H, Q_LEN, D)


# revision 13
# speedup vs baseline: 1.0174x; 1.0174x over previous
"""Trainium2 Bass kernel for masked (block-sparse) attention.

Problem: B=2, H=16, Q=2304, KV=2305, D=64, fp32, with a static structured mask
(policy-transformer causal mask: 8 steps x 288 tokens, 1 sink token, history=4).

Sharding: the 32 (b,h) pairs are split 4-per-core across 8 NeuronCores (pure
data/head parallelism, no collectives).

Per-core algorithm (per head):
  - q/k are transposed on-device (PE transpose) into d-major layout.
  - The mask is folded INTO the QK^T matmul as 24 extra contraction rows:
    mask bias depends on q only through (token-class, step) -- 3 classes x 8
    steps -- so  bias[kv,q] = u[:, kv].T @ w[:, q]  exactly, with u/w small
    host-precomputed tables appended below k^T / q^T.  Masked scores come out
    of the matmul at -240000 and exp(0.125*s) flushes them to exactly 0.
  - S^T chunks [128 kv x 256 q] are computed only for kv-chunks inside each
    q-block's history window (block-sparse skip), exp'd on the scalar engine
    (scale=0.125 applies 1/sqrt(64)), and PV accumulates out^T = [V|1]^T @ P^T
    in PSUM; the appended ones-column of V yields the softmax denominator.
  - Finalize: transpose out^T back with the PE, multiply by the reciprocal
    denominator, DMA out.  No max-subtraction is needed (|scores| <= ~8).
"""

import numpy as np

import concourse.bacc as bacc
import concourse.bass as bass
import concourse.tile as tile
from concourse import mybir
from concourse.bass_utils import run_bass_kernel_spmd

# ---- problem constants -------------------------------------------------------
B, H, D = 2, 16, 64
N_STEPS, N_SINK, HIST = 8, 1, 4
ONE_STEP = 288
TOK_TO_AO = 280
Q_LEN = ONE_STEP * N_STEPS          # 2304
KV_LEN = Q_LEN + N_SINK             # 2305
KV_PAD = 2432                       # 19 * 128
BIG = 30000.0 * 8.0                 # pre-scale mask bias (ACT applies 0.125)
QBLK = 256
CHUNK = 128
N_QBLK = Q_LEN // QBLK              # 9
HEADS_PER_CORE = 4
N_CORES = 8
GROUP = 4                           # S^T chunks per PSUM group / exp call
FP32 = mybir.dt.float32
F32R = mybir.dt.float32r
import os
USE_F32R = os.environ.get("ATTN_F32R", "1") == "1"  # single-pass reduced-precision matmuls
MMDT_G = F32R if USE_F32R else FP32


# ---- host-side mask tables ---------------------------------------------------
def _build_mask():
    q_idx = np.arange(Q_LEN)[:, None]
    kv_idx = np.arange(KV_LEN)[None, :]
    is_sink = kv_idx < N_SINK
    kv_b = kv_idx - N_SINK

    def classify(idx):
        r = idx % ONE_STEP
        return r < TOK_TO_AO, r > TOK_TO_AO, r == TOK_TO_AO, idx // ONE_STEP

    q_img, q_ra, q_ao, q_step = classify(q_idx)
    kv_img, kv_ra, kv_ao, kv_step = classify(kv_b)
    prev = kv_step < q_step
    same = kv_step == q_step
    full = (
        (q_img & ~kv_ao & prev)
        | (q_ao & ~kv_ao & prev)
        | (q_ra & ~kv_ao & prev)
        | (q_img & kv_img & same)
        | (q_ao & (kv_img | kv_ao) & same)
        | (q_ra & ~kv_ao & same)
    )
    full = full & (q_step - kv_step <= HIST)
    return is_sink | full


def _build_uw():
    mask = _build_mask()
    w = np.zeros((24, Q_LEN), np.float32)
    u = np.zeros((24, KV_PAD), np.float32)
    q_r = np.arange(Q_LEN) % ONE_STEP
    q_s = np.arange(Q_LEN) // ONE_STEP
    cls = np.where(q_r < TOK_TO_AO, 0, np.where(q_r == TOK_TO_AO, 1, 2))
    for s in range(N_STEPS):
        for c in range(3):
            row = s * 3 + c
            w[row] = ((q_s == s) & (cls == c)).astype(np.float32)
            rep = s * ONE_STEP + (0, TOK_TO_AO, TOK_TO_AO + 1)[c]
            u[row, :KV_LEN] = np.where(mask[rep], 0.0, -BIG)
            u[row, KV_LEN:] = -BIG
    return u, w


def _chunk_lists():
    blocks = []
    for b in range(N_QBLK):
        r0, r1 = b * QBLK, b * QBLK + QBLK - 1
        s0, s1 = r0 // ONE_STEP, r1 // ONE_STEP
        ws = max(0, s0 - HIST)
        end = ONE_STEP * (s1 + 1) + 1
        c_hi = (end + CHUNK - 1) // CHUNK - 1
        if ws == 0:
            chunks = list(range(0, c_hi + 1))
        else:
            chunks = [0] + list(range((ONE_STEP * ws + 1) // CHUNK, c_hi + 1))
        blocks.append(chunks)
    return blocks


# ---- bass kernel builder -----------------------------------------------------
def _emit(nc, tc, q_d, k_d, v_d, o_d, u_d, w_d, id_d):
    Exp = mybir.ActivationFunctionType.Exp
    blocks = _chunk_lists()

    MMDT = F32R if USE_F32R else FP32

    def mm(out, lhsT, rhs, **kw):
        nc.tensor.matmul(out, lhsT, rhs, **kw)

    import contextlib
    with contextlib.ExitStack() as ctx:
        const_p = ctx.enter_context(tc.tile_pool(name="const", bufs=1))
        big_p = ctx.enter_context(tc.tile_pool(name="big", bufs=2))
        stage_p = ctx.enter_context(tc.tile_pool(name="stage", bufs=2))
        pt_p = ctx.enter_context(tc.tile_pool(name="pt", bufs=3))
        fin_p = ctx.enter_context(tc.tile_pool(name="fin", bufs=2))
        ps_s = ctx.enter_context(tc.tile_pool(name="ps_s", bufs=2, space="PSUM"))
        ps_pv = ctx.enter_context(tc.tile_pool(name="ps_pv", bufs=2, space="PSUM"))
        ps_tr = ctx.enter_context(tc.tile_pool(name="ps_tr", bufs=2, space="PSUM"))

        ident = const_p.tile([128, 128], FP32)
        nc.sync.dma_start(ident[:], id_d.ap()[:])

        def emit_loads(h):
            stq = stage_p.tile([128, 18 * 64], FP32, tag="stq", name=f"stq{h}")
            nc.sync.dma_start(
                stq[:].rearrange("p (t d) -> p t d", d=64),
                q_d.ap()[h].rearrange("(t p) d -> p t d", p=128))
            stk = stage_p.tile([128, 19 * 64], FP32, tag="stk", name=f"stk{h}")
            nc.sync.dma_start(
                stk[:, 0:18 * 64].rearrange("p (t d) -> p t d", d=64),
                k_d.ap()[h, 0:2304].rearrange("(t p) d -> p t d", p=128))
            nc.sync.dma_start(stk[0:1, 18 * 64:19 * 64], k_d.ap()[h, 2304:2305, :])

            qT = big_p.tile([88, Q_LEN], MMDT, tag="qT", name=f"qT{h}")
            kT = big_p.tile([88, KV_PAD], MMDT, tag="kT", name=f"kT{h}")
            nc.sync.dma_start(qT[64:88, :], w_d.ap()[:])
            nc.sync.dma_start(kT[64:88, :], u_d.ap()[:])
            nc.vector.memset(kT[0:64, KV_LEN:KV_PAD].bitcast(FP32), 0.0)

            vsb = big_p.tile([128, 19 * 65], MMDT, tag="vsb", name=f"vsb{h}")
            va = v_d.ap()[h]
            nc.vector.memset(vsb[:, 18 * 65:18 * 65 + 64].bitcast(FP32), 0.0)
            nc.sync.dma_start(
                vsb[:].rearrange("p (c x) -> p c x", x=65)[:, 0:18, 0:64],
                va[0:2304].rearrange("(c p) d -> p c d", p=128))
            nc.sync.dma_start(vsb[0:1, 18 * 65:18 * 65 + 64], va[2304:2305, :])
            ones_view = vsb[:].rearrange("p (c x) -> p c x", x=65)[:, :, 64:65]
            nc.vector.memset(ones_view.bitcast(FP32), 1.0)
            return stq, stk, qT, kT, vsb

        # PE-transpose PAIRS of 128x64 tiles in one [128,128] is_transpose
        # (two pairs per PSUM tile); copies split across ACT and DVE so the
        # PSUM slot frees quickly.
        def load_T2(st, dst, pair_ids):
            tr = ps_tr.tile([128, 256], FP32, tag="tr")
            for j, g in enumerate(pair_ids):
                nc.tensor.transpose(tr[:, j * 128:(j + 1) * 128],
                                    st[:, g * 128:(g + 1) * 128],
                                    ident[:])
            for j, g in enumerate(pair_ids):
                nc.vector.tensor_copy(
                    dst[0:64, (2 * g) * 128:(2 * g) * 128 + 128],
                    tr[0:64, j * 128:(j + 1) * 128])
                nc.vector.tensor_copy(
                    dst[0:64, (2 * g + 1) * 128:(2 * g + 1) * 128 + 128],
                    tr[64:128, j * 128:(j + 1) * 128])

        def transpose_units(tiles):
            stq, stk, qT, kT, vsb = tiles
            units = []
            for g in range(0, 9, 2):
                units.append(lambda g=g: load_T2(stq, qT, [g] if g == 8 else [g, g + 1]))
            for g in range(0, 9, 2):
                units.append(lambda g=g: load_T2(stk, kT, [g] if g == 8 else [g, g + 1]))

            def tiny():
                trl = ps_tr.tile([128, 256], FP32, tag="tr")
                nc.tensor.transpose(trl[0:64, 0:1], stk[0:1, 18 * 64:19 * 64],
                                    ident[0:1, 0:1])
                nc.vector.tensor_copy(kT[0:64, 2304:2305], trl[0:64, 0:1])
            units.append(tiny)
            return units

        def emit_block(h, b, chunks, tiles, osb):
            stq, stk, qT, kT, vsb = tiles
            qTb = qT[:, b * QBLK:(b + 1) * QBLK]
            pv = ps_pv.tile([65, QBLK], FP32, tag="pv")
            n = len(chunks)
            first = True
            for g0 in range(0, n, GROUP):
                grp = chunks[g0:g0 + GROUP]
                sg = ps_s.tile([128, GROUP * QBLK], FP32, tag="sg")
                for j, c in enumerate(grp):
                    mm(sg[:, j * QBLK:(j + 1) * QBLK],
                       kT[:, c * CHUNK:(c + 1) * CHUNK],
                       qTb, start=True, stop=True)
                pt = pt_p.tile([128, GROUP * QBLK], MMDT, tag="pt")
                gw = len(grp) * QBLK
                nc.scalar.activation(pt[:, 0:gw], sg[:, 0:gw], Exp, scale=0.125)
                for j, c in enumerate(grp):
                    last = (g0 + j + 1 == n)
                    mm(pv[:], vsb[:, c * 65:(c + 1) * 65],
                       pt[:, j * QBLK:(j + 1) * QBLK],
                       start=first, stop=last, skip_group_check=True)
                    first = False

            pvs = fin_p.tile([65, QBLK], FP32, tag="pvs")
            nc.vector.tensor_copy(pvs[:], pv[:])
            tf = ps_tr.tile([128, 130], FP32, tag="tr")
            nc.tensor.transpose(tf[:, 0:65], pvs[:, 0:128], ident[0:65, 0:65])
            nc.tensor.transpose(tf[:, 65:130], pvs[:, 128:256], ident[0:65, 0:65])
            rcp = fin_p.tile([128, 2], FP32, tag="rcp")
            nc.vector.reciprocal(rcp[:, 0:1], tf[:, 64:65])
            nc.vector.reciprocal(rcp[:, 1:2], tf[:, 129:130])
            nc.vector.tensor_scalar_mul(osb[:, b * 128:b * 128 + 64],
                                        tf[:, 0:64], rcp[:, 0:1])
            nc.vector.tensor_scalar_mul(osb[:, b * 128 + 64:b * 128 + 128],
                                        tf[:, 65:129], rcp[:, 1:2])

        # software-pipelined head loop: head h+1's loads/transposes are
        # interleaved between head h's blocks so its setup hides under
        # chunk compute.
        tiles = emit_loads(0)
        for unit in transpose_units(tiles):
            unit()
        for h in range(HEADS_PER_CORE):
            nxt_units = []
            if h + 1 < HEADS_PER_CORE:
                nxt_tiles = emit_loads(h + 1)
                nxt_units = transpose_units(nxt_tiles)
            osb = fin_p.tile([128, 18 * 64], FP32, tag="osb", name=f"osb{h}")
            per = max(1, (len(nxt_units) + N_QBLK - 1) // N_QBLK)
            for b, chunks in enumerate(blocks):
                emit_block(h, b, chunks, tiles, osb)
                for _ in range(per):
                    if nxt_units:
                        nxt_units.pop(0)()
            while nxt_units:
                nxt_units.pop(0)()
            nc.sync.dma_start(
                o_d.ap()[h].rearrange("(t p) d -> p t d", p=128),
                osb[:].rearrange("p (t d) -> p t d", d=64))
            if h + 1 < HEADS_PER_CORE:
                tiles = nxt_tiles


_CACHE = {}


def _get_nc():
    if "nc" not in _CACHE:
        nc = bacc.Bacc("TRN2", target_bir_lowering=False, debug=False)
        q_d = nc.dram_tensor("q", [HEADS_PER_CORE, Q_LEN, D], FP32, kind="ExternalInput")
        k_d = nc.dram_tensor("k", [HEADS_PER_CORE, KV_LEN, D], FP32, kind="ExternalInput")
        v_d = nc.dram_tensor("v", [HEADS_PER_CORE, KV_LEN, D], MMDT_G, kind="ExternalInput")
        u_d = nc.dram_tensor("utab", [24, KV_PAD], MMDT_G, kind="ExternalInput")
        w_d = nc.dram_tensor("wtab", [24, Q_LEN], MMDT_G, kind="ExternalInput")
        id_d = nc.dram_tensor("ident", [128, 128], FP32, kind="ExternalInput")
        o_d = nc.dram_tensor("o", [HEADS_PER_CORE, Q_LEN, D], FP32, kind="ExternalOutput")
        with tile.TileContext(nc) as tc:
            _emit(nc, tc, q_d, k_d, v_d, o_d, u_d, w_d, id_d)
        nc.compile()
        _CACHE["nc"] = nc
    return _CACHE["nc"]


LAST_RESULT = None


def kernel(q, k, v):
    global LAST_RESULT
    q = np.ascontiguousarray(np.asarray(q, np.float32).reshape(B * H, Q_LEN, D))
    k = np.ascontiguousarray(np.asarray(k, np.float32).reshape(B * H, KV_LEN, D))
    v = np.ascontiguousarray(np.asarray(v, np.float32).reshape(B * H, KV_LEN, D))

    u, w = _build_uw()
    ident = np.eye(128, dtype=np.float32)

    nc = _get_nc()
    in_maps = []
    for c in range(N_CORES):
        s = slice(c * HEADS_PER_CORE, (c + 1) * HEADS_PER_CORE)
        in_maps.append({
            "q": q[s], "k": k[s], "v": v[s],
            "utab": u, "wtab": w, "ident": ident,
        })
    res = run_bass_kernel_spmd(nc, in_maps, list(range(N_CORES)))
    LAST_RESULT = res
    out = np.concatenate([res.results[c]["o"] for c in range(N_CORES)], axis=0)
    return out.reshape(B, H, Q_LEN, D)


# revision 14
# speedup vs baseline: 1.1089x; 1.0899x over previous
"""Trainium2 Bass kernel for masked (block-sparse) attention.

Problem: B=2, H=16, Q=2304, KV=2305, D=64, fp32, with a static structured mask
(policy-transformer causal mask: 8 steps x 288 tokens, 1 sink token, history=4).

Sharding: the 32 (b,h) pairs are split 4-per-core across 8 NeuronCores (pure
data/head parallelism, no collectives).

Per-core algorithm (per head):
  - q/k are transposed on-device (PE transpose) into d-major layout.
  - The mask is folded INTO the QK^T matmul as 24 extra contraction rows:
    mask bias depends on q only through (token-class, step) -- 3 classes x 8
    steps -- so  bias[kv,q] = u[:, kv].T @ w[:, q]  exactly, with u/w small
    host-precomputed tables appended below k^T / q^T.  Masked scores come out
    of the matmul at -240000 and exp(0.125*s) flushes them to exactly 0.
  - S^T chunks [128 kv x 256 q] are computed only for kv-chunks inside each
    q-block's history window (block-sparse skip), exp'd on the scalar engine
    (scale=0.125 applies 1/sqrt(64)), and PV accumulates out^T = [V|1]^T @ P^T
    in PSUM; the appended ones-column of V yields the softmax denominator.
  - Finalize: transpose out^T back with the PE, multiply by the reciprocal
    denominator, DMA out.  No max-subtraction is needed (|scores| <= ~8).
"""

import numpy as np

import concourse.bacc as bacc
import concourse.bass as bass
import concourse.tile as tile
from concourse import mybir
from concourse.bass_utils import run_bass_kernel_spmd

# ---- problem constants -------------------------------------------------------
B, H, D = 2, 16, 64
N_STEPS, N_SINK, HIST = 8, 1, 4
ONE_STEP = 288
TOK_TO_AO = 280
Q_LEN = ONE_STEP * N_STEPS          # 2304
KV_LEN = Q_LEN + N_SINK             # 2305
KV_PAD = 2432                       # 19 * 128
BIG = 30000.0 * 8.0                 # pre-scale mask bias (ACT applies 0.125)
QBLK = 256
CHUNK = 128
N_QBLK = Q_LEN // QBLK              # 9
HEADS_PER_CORE = 4
N_CORES = 8
GROUP = 4                           # S^T chunks per PSUM group / exp call
FP32 = mybir.dt.float32
F32R = mybir.dt.float32r
import os
USE_F32R = os.environ.get("ATTN_F32R", "1") == "1"  # single-pass reduced-precision matmuls
MMDT_G = F32R if USE_F32R else FP32


# ---- host-side mask tables ---------------------------------------------------
def _build_mask():
    q_idx = np.arange(Q_LEN)[:, None]
    kv_idx = np.arange(KV_LEN)[None, :]
    is_sink = kv_idx < N_SINK
    kv_b = kv_idx - N_SINK

    def classify(idx):
        r = idx % ONE_STEP
        return r < TOK_TO_AO, r > TOK_TO_AO, r == TOK_TO_AO, idx // ONE_STEP

    q_img, q_ra, q_ao, q_step = classify(q_idx)
    kv_img, kv_ra, kv_ao, kv_step = classify(kv_b)
    prev = kv_step < q_step
    same = kv_step == q_step
    full = (
        (q_img & ~kv_ao & prev)
        | (q_ao & ~kv_ao & prev)
        | (q_ra & ~kv_ao & prev)
        | (q_img & kv_img & same)
        | (q_ao & (kv_img | kv_ao) & same)
        | (q_ra & ~kv_ao & same)
    )
    full = full & (q_step - kv_step <= HIST)
    return is_sink | full


def _build_uw():
    mask = _build_mask()
    w = np.zeros((24, Q_LEN), np.float32)
    u = np.zeros((24, KV_PAD), np.float32)
    q_r = np.arange(Q_LEN) % ONE_STEP
    q_s = np.arange(Q_LEN) // ONE_STEP
    cls = np.where(q_r < TOK_TO_AO, 0, np.where(q_r == TOK_TO_AO, 1, 2))
    for s in range(N_STEPS):
        for c in range(3):
            row = s * 3 + c
            w[row] = ((q_s == s) & (cls == c)).astype(np.float32)
            rep = s * ONE_STEP + (0, TOK_TO_AO, TOK_TO_AO + 1)[c]
            u[row, :KV_LEN] = np.where(mask[rep], 0.0, -BIG)
            u[row, KV_LEN:] = -BIG
    return u, w


def _chunk_lists():
    blocks = []
    for b in range(N_QBLK):
        r0, r1 = b * QBLK, b * QBLK + QBLK - 1
        s0, s1 = r0 // ONE_STEP, r1 // ONE_STEP
        ws = max(0, s0 - HIST)
        end = ONE_STEP * (s1 + 1) + 1
        c_hi = (end + CHUNK - 1) // CHUNK - 1
        if ws == 0:
            chunks = list(range(0, c_hi + 1))
        else:
            chunks = [0] + list(range((ONE_STEP * ws + 1) // CHUNK, c_hi + 1))
        blocks.append(chunks)
    return blocks


# ---- bass kernel builder -----------------------------------------------------
def _emit(nc, tc, q_d, k_d, v_d, o_d, u_d, w_d, id_d):
    Exp = mybir.ActivationFunctionType.Exp
    blocks = _chunk_lists()

    MMDT = F32R if USE_F32R else FP32

    def mm(out, lhsT, rhs, **kw):
        nc.tensor.matmul(out, lhsT, rhs, **kw)

    import contextlib
    with contextlib.ExitStack() as ctx:
        const_p = ctx.enter_context(tc.tile_pool(name="const", bufs=1))
        big_p = ctx.enter_context(tc.tile_pool(name="big", bufs=2))
        stage_p = ctx.enter_context(tc.tile_pool(name="stage", bufs=2))
        pt_p = ctx.enter_context(tc.tile_pool(name="pt", bufs=3))
        fin_p = ctx.enter_context(tc.tile_pool(name="fin", bufs=2))
        ps_s = ctx.enter_context(tc.tile_pool(name="ps_s", bufs=2, space="PSUM"))
        ps_pv = ctx.enter_context(tc.tile_pool(name="ps_pv", bufs=2, space="PSUM"))
        ps_tr = ctx.enter_context(tc.tile_pool(name="ps_tr", bufs=2, space="PSUM"))

        ident = const_p.tile([128, 128], FP32)
        nc.sync.dma_start(ident[:], id_d.ap()[:])

        for h in range(HEADS_PER_CORE):
            # ---- staged natural-layout loads (batched DMAs) ----
            stq = stage_p.tile([128, 18 * 64], FP32, tag="stq")
            nc.sync.dma_start(
                stq[:].rearrange("p (t d) -> p t d", d=64),
                q_d.ap()[h].rearrange("(t p) d -> p t d", p=128))
            stk = stage_p.tile([128, 19 * 64], FP32, tag="stk")
            nc.sync.dma_start(
                stk[:, 0:18 * 64].rearrange("p (t d) -> p t d", d=64),
                k_d.ap()[h, 0:2304].rearrange("(t p) d -> p t d", p=128))
            nc.sync.dma_start(stk[0:1, 18 * 64:19 * 64], k_d.ap()[h, 2304:2305, :])

            # ---- transposed / augmented operands ----
            qT = big_p.tile([88, Q_LEN], MMDT, tag="qT")
            kT = big_p.tile([88, KV_PAD], MMDT, tag="kT")

            nc.sync.dma_start(qT[64:88, :], w_d.ap()[:])
            nc.sync.dma_start(kT[64:88, :], u_d.ap()[:])
            nc.vector.memset(kT[0:64, KV_LEN:KV_PAD].bitcast(FP32), 0.0)

            # PE-transpose 128x64 tiles in batches of 4 -> [64, 512] PSUM ->
            # one ACT copy per batch into the d-major tile.
            def load_T(st, dst, tile_ids):
                tr = ps_tr.tile([64, 512], FP32, tag="tr")
                for j, t in enumerate(tile_ids):
                    nc.tensor.transpose(tr[:, j * 128:(j + 1) * 128],
                                        st[:, t * 64:(t + 1) * 64],
                                        ident[:])
                wdt = len(tile_ids) * 128
                nc.scalar.copy(dst[0:64, tile_ids[0] * 128:tile_ids[0] * 128 + wdt],
                               tr[:, 0:wdt])

            for g in range(5):
                load_T(stq, qT, list(range(g * 4, min(g * 4 + 4, 18))))
            for g in range(5):
                load_T(stk, kT, list(range(g * 4, min(g * 4 + 4, 18))))
            # k row 2304 (tile 18, partition 0 of stage)
            trl = ps_tr.tile([64, 512], FP32, tag="tr")
            nc.tensor.transpose(trl[:, 0:1], stk[0:1, 18 * 64:19 * 64], ident[0:1, 0:1])
            nc.scalar.copy(kT[0:64, 2304:2305], trl[:, 0:1])

            # ---- V (natural layout) + ones column ----
            vsb = big_p.tile([128, 19 * 65], MMDT, tag="vsb")
            va = v_d.ap()[h]
            nc.vector.memset(vsb[:, 18 * 65:18 * 65 + 64].bitcast(FP32), 0.0)
            nc.sync.dma_start(
                vsb[:].rearrange("p (c x) -> p c x", x=65)[:, 0:18, 0:64],
                va[0:2304].rearrange("(c p) d -> p c d", p=128))
            nc.sync.dma_start(vsb[0:1, 18 * 65:18 * 65 + 64], va[2304:2305, :])
            ones_view = vsb[:].rearrange("p (c x) -> p c x", x=65)[:, :, 64:65]
            nc.vector.memset(ones_view.bitcast(FP32), 1.0)

            # ---- blocks ----
            osb = fin_p.tile([128, 18 * 64], FP32, tag="osb")
            for b, chunks in enumerate(blocks):
                qTb = qT[:, b * QBLK:(b + 1) * QBLK]
                pv = ps_pv.tile([65, QBLK], FP32, tag="pv")
                n = len(chunks)
                first = True
                for g0 in range(0, n, GROUP):
                    grp = chunks[g0:g0 + GROUP]
                    sg = ps_s.tile([128, GROUP * QBLK], FP32, tag="sg")
                    for j, c in enumerate(grp):
                        mm(sg[:, j * QBLK:(j + 1) * QBLK],
                           kT[:, c * CHUNK:(c + 1) * CHUNK],
                           qTb, start=True, stop=True)
                    pt = pt_p.tile([128, GROUP * QBLK], MMDT, tag="pt")
                    gw = len(grp) * QBLK
                    nc.scalar.activation(pt[:, 0:gw], sg[:, 0:gw], Exp, scale=0.125)
                    for j, c in enumerate(grp):
                        last = (g0 + j + 1 == n)
                        mm(pv[:], vsb[:, c * 65:(c + 1) * 65],
                           pt[:, j * QBLK:(j + 1) * QBLK],
                           start=first, stop=last, skip_group_check=True)
                        first = False

                # ---- finalize block: transpose back, normalize, store ----
                pvs = fin_p.tile([65, QBLK], FP32, tag="pvs")
                nc.vector.tensor_copy(pvs[:], pv[:])
                tf = ps_tr.tile([128, 130], FP32, tag="tr")
                nc.tensor.transpose(tf[:, 0:65], pvs[:, 0:128], ident[0:65, 0:65])
                nc.tensor.transpose(tf[:, 65:130], pvs[:, 128:256], ident[0:65, 0:65])
                rcp = fin_p.tile([128, 2], FP32, tag="rcp")
                nc.vector.reciprocal(rcp[:, 0:1], tf[:, 64:65])
                nc.vector.reciprocal(rcp[:, 1:2], tf[:, 129:130])
                nc.vector.tensor_scalar_mul(osb[:, b * 128:b * 128 + 64],
                                            tf[:, 0:64], rcp[:, 0:1])
                nc.vector.tensor_scalar_mul(osb[:, b * 128 + 64:b * 128 + 128],
                                            tf[:, 65:129], rcp[:, 1:2])

            nc.sync.dma_start(
                o_d.ap()[h].rearrange("(t p) d -> p t d", p=128),
                osb[:].rearrange("p (t d) -> p t d", d=64))


_CACHE = {}


def _get_nc():
    if "nc" not in _CACHE:
        nc = bacc.Bacc("TRN2", target_bir_lowering=False, debug=False)
        q_d = nc.dram_tensor("q", [HEADS_PER_CORE, Q_LEN, D], FP32, kind="ExternalInput")
        k_d = nc.dram_tensor("k", [HEADS_PER_CORE, KV_LEN, D], FP32, kind="ExternalInput")
        v_d = nc.dram_tensor("v", [HEADS_PER_CORE, KV_LEN, D], MMDT_G, kind="ExternalInput")
        u_d = nc.dram_tensor("utab", [24, KV_PAD], MMDT_G, kind="ExternalInput")
        w_d = nc.dram_tensor("wtab", [24, Q_LEN], MMDT_G, kind="ExternalInput")
        id_d = nc.dram_tensor("ident", [128, 128], FP32, kind="ExternalInput")
        o_d = nc.dram_tensor("o", [HEADS_PER_CORE, Q_LEN, D], FP32, kind="ExternalOutput")
        with tile.TileContext(nc) as tc:
            _emit(nc, tc, q_d, k_d, v_d, o_d, u_d, w_d, id_d)
        nc.compile()
        _CACHE["nc"] = nc
    return _CACHE["nc"]


LAST_RESULT = None


def kernel(q, k, v):
    global LAST_RESULT
    q = np.ascontiguousarray(np.asarray(q, np.float32).reshape(B * H, Q_LEN, D))
    k = np.ascontiguousarray(np.asarray(k, np.float32).reshape(B * H, KV_LEN, D))
    v = np.ascontiguousarray(np.asarray(v, np.float32).reshape(B * H, KV_LEN, D))

    u, w = _build_uw()
    ident = np.eye(128, dtype=np.float32)

    nc = _get_nc()
    in_maps = []
    for c in range(N_CORES):
        s = slice(c * HEADS_PER_CORE, (c + 1) * HEADS_PER_CORE)
        in_maps.append({
            "q": q[s], "k": k[s], "v": v[s],
            "utab": u, "wtab": w, "ident": ident,
        })
    res = run_bass_kernel_spmd(nc, in_maps, list(range(N_CORES)))
    LAST_RESULT = res
    out = np.concatenate([res.results[c]["o"] for c in range(N_CORES)], axis=0)
    return out.reshape(B, H, Q_LEN, D)
